# revision 1
# baseline (speedup 1.0000x reference)
"""BSM (bipartite soft matching) token-merge kernel for Trainium2.

Data-parallel over the batch dim: 64 batch rows are split 8-per-core
across 8 NeuronCores; each core runs an identical Bass program
(software-pipelined: phase B of row b-1 is emitted after phase A of
row b).

Per batch row:
  phase A (compute + index build):
    scores = a @ b.T        (PE, fp32)   a=k[b,::2,:], b=k[b,1::2,:]
    node_max / node_idx     (DVE max8 + max_index per 128-row chunk)
    vb broadcast            (PE transposes -> Act copy -> gpsimd bcast)
    rank[i] = #{j<c0: v[j]>=v[i]} + #{j>=c0: v[j]>v[i]}
            + #{j<i in chunk: v[j]==v[i]}      (sliced DVE ts + stt)
    perm/dstv by rank       (gpsimd local_scatter at wrapped positions,
                             DRAM bounce, replicated index tiles)
    one-hot merge matrix M  (Pool tensor_scalar vs iota, bf16)
  phase B (payload):
    dst tokens copied DRAM->DRAM to out rows 256..767;
    dma_gather unmerged ranks 256..511 -> out rows 0..255 (plain DMA);
    dma_gather merged ranks 0..255, bf16 one-hot matmul per dst chunk
    (collision-proof per-dst sums), PSUM->SBUF copies, then SWDGE
    accumulate-DMA (CCE fp32 add, unique rows) onto the dst region.
"""

import sys
from contextlib import ExitStack

for _p in ("/root/.axon_site/_ro/trn_rl_repo", "/opt/trn_rl_repo"):
    if _p not in sys.path:
        sys.path.append(_p)

import numpy as np  # noqa: E402

from concourse import bacc, bass, tile  # noqa: E402
from concourse import mybir  # noqa: E402
from concourse.bass_utils import run_bass_kernel_spmd  # noqa: E402

DT = mybir.dt
F32 = DT.float32
I16 = DT.int16
U16 = DT.uint16
BF16 = DT.bfloat16
ALU = mybir.AluOpType
AX = mybir.AxisListType

B, T, C, CK, R = 64, 1024, 768, 64, 256
NCORES = 8
BL = B // NCORES          # 8 batch rows per core
TH = T // 2               # 512 source (and dst) tokens
NU = TH - R               # 256 unmerged tokens
NCH = TH // 128           # 4 chunks of 128 source tokens

NEG_INF = -1e30


def build_nc(bl: int = BL, debug: bool = False):
    nc = bacc.Bacc("TRN2", target_bir_lowering=False, debug=debug)
    x = nc.dram_tensor("x", [bl, T, C], F32, kind="ExternalInput")
    k = nc.dram_tensor("k", [bl, T, CK], F32, kind="ExternalInput")
    out = nc.dram_tensor("out", [bl, T - R, C], F32, kind="ExternalOutput")

    with tile.TileContext(nc) as tc:
        emit(tc, out.ap(), x.ap(), k.ap(), bl)

    nc.compile()
    return nc


def emit(tc: tile.TileContext, out: bass.AP, x: bass.AP, k: bass.AP, bl: int):
    nc = tc.nc
    ctx = ExitStack()
    with ctx:
        const = ctx.enter_context(tc.tile_pool(name="const", bufs=1))
        kraw_p = ctx.enter_context(tc.tile_pool(name="kraw", bufs=3))
        kt_p = ctx.enter_context(tc.tile_pool(name="kt", bufs=3))
        small_p = ctx.enter_context(tc.tile_pool(name="small", bufs=3))
        scr_p = ctx.enter_context(tc.tile_pool(name="scr", bufs=3))
        ls_p = ctx.enter_context(tc.tile_pool(name="ls", bufs=1))
        idx_p = ctx.enter_context(tc.tile_pool(name="idx", bufs=3))
        g_p = ctx.enter_context(tc.tile_pool(name="g", bufs=3))
        vb_p = ctx.enter_context(tc.tile_pool(name="vb", bufs=3))
        dram_p = ctx.enter_context(tc.tile_pool(name="dram", bufs=3,
                                                space="DRAM"))
        m_p = ctx.enter_context(tc.tile_pool(name="m", bufs=3))
        sb_p = ctx.enter_context(tc.tile_pool(name="sbp", bufs=2))
        ps_score = ctx.enter_context(
            tc.tile_pool(name="ps_score", bufs=2, space="PSUM"))
        ps_tr = ctx.enter_context(
            tc.tile_pool(name="ps_tr", bufs=2, space="PSUM"))
        ps_s = ctx.enter_context(
            tc.tile_pool(name="ps_s", bufs=2, space="PSUM"))

        # ---- constants ----
        ones_sq = const.tile([128, 128], F32)
        nc.vector.memset(ones_sq[:], 1.0)
        ident = const.tile([128, 128], F32)        # PE transpose identity
        nc.gpsimd.affine_select(
            ident[:], ones_sq[:], pattern=[[-1, 128]], base=0,
            channel_multiplier=1, compare_op=ALU.is_equal, fill=0.0)
        # diagonal-block triangle: triD[i, j] = 1 if j < i else 0
        triD = const.tile([128, 128], F32, tag="triD")
        nc.gpsimd.affine_select(
            triD[:], ones_sq[:], pattern=[[-1, 128]], base=0,
            channel_multiplier=1, compare_op=ALU.is_gt, fill=0.0)
        tri = [triD]
        iota_bc = const.tile([128, TH], F32)       # 0..511 on every partition
        nc.gpsimd.iota(iota_bc[:], pattern=[[1, TH]], base=0,
                       channel_multiplier=0,
                       allow_small_or_imprecise_dtypes=True)
        # sigma-order token ids: data_iota[0, 4*p + c] = p + 128*c
        data_iota = const.tile([1, TH], I16)
        nc.gpsimd.iota(data_iota[:], pattern=[[1, 128], [128, NCH]], base=0,
                       channel_multiplier=0)

        # local_scatter in/out tiles (bufs=1: rows 2..15 of ls_idx stay -1,
        # row 0 of ls_dat stays data_iota; only per-row slots rewritten)
        ls_idx = ls_p.tile([16, TH], I16, tag="ls_idx")
        ls_dat = ls_p.tile([16, TH], I16, tag="ls_dat")
        nc.vector.memset(ls_idx[:, :], -1)
        nc.vector.memset(ls_dat[:, :], 0)
        nc.vector.tensor_copy(ls_dat[0:1, :], data_iota[:])

        # software pipeline: phase B (payload) of row b-1 is emitted
        # after phase A of row b, so per-engine in-order queues never
        # block row b's index chain behind row b-1's bulk DMA work.
        pend = None
        for b in range(bl):
            st = _emit_phase_a(tc, nc, out, x, k, b,
                               ident, tri, iota_bc,
                               ls_idx, ls_dat,
                               kraw_p, kt_p, small_p, scr_p, idx_p, g_p,
                               vb_p, m_p, dram_p, ps_score, ps_tr)
            if pend is not None:
                _emit_phase_b(tc, nc, out, x, *pend,
                              g_p, sb_p, ps_s)
            pend = (b, st)
        _emit_phase_b(tc, nc, out, x, *pend, g_p, sb_p, ps_s)


def _emit_phase_a(tc, nc, out, x, k, b,
                  ident, tri, iota_bc, ls_idx, ls_dat,
                  kraw_p, kt_p, small_p, scr_p, idx_p, g_p,
                  vb_p, m_p, dram_p, ps_score, ps_tr):
    # ---- load k and transpose to [CK, TH] for each half ----
    kb = k[b]                                   # [T, CK]
    khalf = kb.rearrange("(t two) c -> two t c", two=2)   # [2, TH, CK]
    kraw = kraw_p.tile([128, 2, NCH, CK], F32, tag="kraw")
    for h in (1, 0):  # kbT first: scores need all of kbT but only kaT[mc]
        src = khalf[h].rearrange("(m p) c -> p m c", p=128)  # [128, NCH, CK]
        nc.sync.dma_start(kraw[:, h, :, :], src)
    kt = kt_p.tile([CK, 2, TH], F32, tag="kt")  # kaT | kbT
    for h in (1, 0):
        ps_t = ps_tr.tile([CK, TH], F32, tag="ps_t")
        for mc in range(NCH):
            nc.tensor.transpose(ps_t[:, mc * 128:(mc + 1) * 128],
                                kraw[:, h, mc, :], ident[:])
        nc.scalar.copy(kt[:, h, :], ps_t[:])

    # ---- scores + node_max + argmax per 128-row chunk ----
    # node_max scalars live in m8[:, mc, 0]; token-0 slot overridden to
    # -inf right after its argmax (protect first src token)
    m8 = small_p.tile([128, NCH, 8], F32, tag="m8")
    i8 = small_p.tile([128, NCH, 8], U16, tag="i8")
    ps_vrow = ps_tr.tile([1, TH], F32, tag="ps_vrow")
    for mc in range(NCH):
        ps = ps_score.tile([128, TH], F32, tag="ps")
        nc.tensor.matmul(ps[:], kt[:, 0, mc * 128:(mc + 1) * 128],
                         kt[:, 1, :], start=True, stop=True)
        nc.vector.max(m8[:, mc, :], ps[:])
        nc.vector.max_index(i8[:, mc, :], m8[:, mc, :], ps[:])
        if mc == 0:
            nc.vector.memset(m8[0:1, 0, 0:1], NEG_INF)
        nc.tensor.transpose(ps_vrow[:, mc * 128:(mc + 1) * 128],
                            m8[:, mc, 0:1], ident[:])

    # ---- broadcast node_max along partitions: vb[i, j] = v[j] ----
    vrow = small_p.tile([1, TH], F32, tag="vrow")
    nc.scalar.copy(vrow[:], ps_vrow[:])
    vb = vb_p.tile([128, TH], F32, tag="vb")
    nc.gpsimd.partition_broadcast(vb[:, :], vrow[:, :])

    # ---- rank[i] = #{j<c0: v[j]>=v[i]} + #{j>=c0: v[j]>v[i]}
    #             + #{j<i, same chunk: v[j]==v[i]}   (c0 = chunk start) ----
    gt_s = small_p.tile([128, NCH], F32, tag="gt_s")
    ge_s = small_p.tile([128, NCH], F32, tag="ge_s")
    td_s = small_p.tile([128, NCH], F32, tag="td_s")
    nc.vector.memset(ge_s[:, 0:1], 0.0)
    for mc in range(NCH):
        c0 = 128 * mc
        junk2 = scr_p.tile([128, TH], F32, tag="junk2")
        nc.vector.tensor_scalar(junk2[:, 0:TH - c0], vb[:, c0:TH],
                                m8[:, mc, 0:1], None,
                                op0=ALU.is_gt, op1=ALU.add,
                                accum_out=gt_s[:, mc:mc + 1])
        if mc > 0:
            nc.vector.tensor_scalar(junk2[:, 0:c0], vb[:, 0:c0],
                                    m8[:, mc, 0:1], None,
                                    op0=ALU.is_ge, op1=ALU.add,
                                    accum_out=ge_s[:, mc:mc + 1])
        eqtri = scr_p.tile([128, 128], F32, tag="eqtri")
        nc.vector.scalar_tensor_tensor(
            eqtri[:], vb[:, c0:c0 + 128], m8[:, mc, 0:1], tri[0][:],
            op0=ALU.is_equal, op1=ALU.mult,
            accum_out=td_s[:, mc:mc + 1])
    # ---- rank = gt + ge + triD, cast to i16 ----
    rank_f = small_p.tile([128, NCH], F32, tag="rank_f")
    nc.vector.tensor_tensor(rank_f[:], gt_s[:], ge_s[:], op=ALU.add)
    rank16 = small_p.tile([128, NCH], I16, tag="rank16")
    nc.vector.tensor_tensor(rank16[:], rank_f[:], td_s[:], op=ALU.add)

    nidx16 = small_p.tile([128, NCH], I16, tag="nidx16")
    nc.vector.tensor_copy(nidx16[:], i8[:, :, 0])

    # ---- wrapped positions w = (r%16)*32 + r//16 ----
    rw1 = small_p.tile([128, NCH], I16, tag="rw1")
    nc.vector.tensor_scalar(rw1[:], rank16[:], 15, 5,
                            op0=ALU.bitwise_and, op1=ALU.logical_shift_left)
    rw2 = small_p.tile([128, NCH], I16, tag="rw2")
    nc.vector.tensor_scalar(rw2[:], rank16[:], 4, None,
                            op0=ALU.logical_shift_right)
    rankw = small_p.tile([128, NCH], I16, tag="rankw")
    nc.vector.tensor_tensor(rankw[:], rw1[:], rw2[:], op=ALU.bitwise_or)

    # ---- local_scatter at wrapped positions (rows 0 and 1 share idx) ----
    ls_out = idx_p.tile([16, TH], I16, tag="ls_out")
    nc.sync.dma_start(ls_idx[0:1, :], rankw[:, :])
    nc.scalar.dma_start(ls_idx[1:2, :], rankw[:, :])
    nc.sync.dma_start(ls_dat[1:2, :], nidx16[:, :])
    nc.gpsimd.local_scatter(ls_out[:], ls_dat[:], ls_idx[:],
                            channels=16, num_elems=TH, num_idxs=TH)

    # ---- bounce wrapped rows to DRAM, replicate to 8 core groups ----
    bounce = dram_p.tile([2, TH], I16, tag="bounce")
    nc.sync.dma_start(bounce[:, :], ls_out[0:2, :])
    bap = bounce[:]
    g_idx = idx_p.tile([128, TH // 16], I16, tag="g_idx")
    rep = bass.AP(bap.tensor, bap.offset,
                  [[0, 8], [TH // 16, 16], [1, TH // 16]])
    nc.sync.dma_start(g_idx[:, :], rep)

    # ---- dst tokens by rank: dstv16[p, kc] = ls_out[1, w32(kc*128+p)] ----
    # w32(q) = (q%16)*32 + q//16 = 32*(p%16) + 8*kc + p//16
    dstv16 = small_p.tile([128, 2], I16, tag="dstv16")
    for kc in range(2):
        dsrc = bass.AP(bap.tensor, bap.offset + TH + 8 * kc,
                       [[1, 8], [32, 16], [1, 1]])
        nc.sync.dma_start(dstv16[:, kc:kc + 1], dsrc)
    dstv_f = small_p.tile([128, 2], F32, tag="dstv_f")
    nc.vector.tensor_copy(dstv_f[:], dstv16[:])
    # one-hot merge matrices (bf16; 0/1 exact) on Pool
    M = m_p.tile([128, 2, TH], BF16, tag="M")
    for kc in range(2):
        nc.gpsimd.tensor_scalar(M[:, kc, :], iota_bc[:],
                                dstv_f[:, kc:kc + 1], None,
                                op0=ALU.is_equal)

    return g_idx, M


def _emit_phase_b(tc, nc, out, x, b, st, g_p, sb_p, ps_s):
    g_idx, M = st
    xb = x[b]                                    # [T, C]
    xhalf = xb.rearrange("(t two) c -> two t c", two=2)  # [2, TH, C]
    x_even = xhalf[0]                            # src rows, stride 2C
    x_odd = xhalf[1]                             # dst rows

    # dst tokens -> out rows NU..T-R (independent of everything above);
    # split across both HWDGE queues to halve per-queue transfer time
    nc.sync.dma_start(out[b, NU:NU + TH // 2, :], x_odd[0:TH // 2])
    nc.scalar.dma_start(out[b, NU + TH // 2:T - R, :], x_odd[TH // 2:TH])

    # gather src rows in rank order: G[p, m, :] = rank 128*m + p
    G = g_p.tile([128, NCH, C], F32, tag="G")
    nc.gpsimd.dma_gather(G[:, 2:4, :], x_even, g_idx[:, 16:32],
                         num_idxs=R, num_idxs_reg=R,
                         elem_size=C, elem_step=2 * C)
    # unmerged rows (rank 256..511) -> out rows 0..255
    nc.sync.dma_start(out[b, 0:128, :], G[:, 2, :])
    nc.sync.dma_start(out[b, 128:256, :], G[:, 3, :])
    nc.gpsimd.dma_gather(G[:, 0:2, :], x_even, g_idx[:, 0:16],
                         num_idxs=R, num_idxs_reg=R,
                         elem_size=C, elem_step=2 * C)

    # merged rows (rank 0..255): one-hot bf16 matmul per dst chunk,
    # copy PSUM->SBUF, then one SWDGE accumulate-DMA onto the dst rows
    # (unique rows -> no RMW races)
    Gb = sb_p.tile([128, 2, C], BF16, tag="Gb")
    for kc in range(2):
        nc.scalar.copy(Gb[:, kc, :], G[:, kc, :])
    Ssb = sb_p.tile([128, NCH, C], F32, tag="Ssb")
    NH = C // 2
    for dc in range(NCH):
        for nh in range(2):
            S = ps_s.tile([128, NH], F32, tag="S")
            for kc in range(2):
                nc.tensor.matmul(S[:], M[:, kc, dc * 128:(dc + 1) * 128],
                                 Gb[:, kc, nh * NH:(nh + 1) * NH],
                                 start=(kc == 0), stop=(kc == 1))
            if nh == 0 or dc >= 2:
                nc.scalar.copy(Ssb[:, dc, nh * NH:(nh + 1) * NH], S[:])
            else:
                nc.vector.tensor_copy(Ssb[:, dc, nh * NH:(nh + 1) * NH],
                                      S[:])
    for half in range(2):
        lo = half * 2
        acc_dst = out[b, NU + lo * 128:NU + (lo + 2) * 128, :].rearrange(
            "(m p) c -> p m c", p=128)
        nc.gpsimd.dma_start(acc_dst, Ssb[:, lo:lo + 2, :],
                            accum_op=ALU.add)


_NC_CACHE = {}


def _get_nc():
    if "nc" not in _NC_CACHE:
        _NC_CACHE["nc"] = build_nc()
    return _NC_CACHE["nc"]


def kernel(x=None, k=None, r=None, _trace=False, **_ignored):
    x = np.ascontiguousarray(np.asarray(x, dtype=np.float32))
    k = np.ascontiguousarray(np.asarray(k, dtype=np.float32))
    rv = int(np.asarray(r)) if r is not None else R
    assert rv == R, f"kernel compiled for r={R}, got r={rv}"
    assert x.shape == (B, T, C) and k.shape == (B, T, CK)

    nc = _get_nc()
    in_maps = [
        {"x": x[i * BL:(i + 1) * BL], "k": k[i * BL:(i + 1) * BL]}
        for i in range(NCORES)
    ]
    res = run_bass_kernel_spmd(nc, in_maps, list(range(NCORES)),
                               trace=_trace)
    outs = [np.asarray(res.results[i]["out"]) for i in range(NCORES)]
    full = np.concatenate(outs, axis=0).astype(np.float32, copy=False)
    if _trace:
        return full, res
    return full

